# revision 1
# baseline (speedup 1.0000x reference)
"""Depth-aware 3x3 convolution on 8 Trainium2 NeuronCores (Bass, raw engine blocks).

out[b,o,h,w] = sum_{c,kh,kw} weight[o,c,kh,kw] * x[b,c,h+kh-1,w+kw-1]
                             * exp(-8.3*|depth[b,h,w] - depth[b,h+kh-1,w+kw-1]|)

Sharding: core = 2*b + (h >= 128); each core computes a [32, 128, 256] output
slab from a 130-row padded input frame (1-row halo from the host slice).

Per-core pipeline:
  A. sim: depth rows pixel-major [128, 258]x3 -> |dc-dk| (DVE) -> exp (ACT, bf16)
     -> DRAM simd[9, 32768]
  B. main loop over 16 tiles of 2048 px (8 rows):
     - DMA: x3 chunk [96, 10*258] (3 column-shift blocks stacked on partitions)
     - DMA: broadcast simd rows across 32 partitions -> simrep3 [96, 2048] bf16
     - DVE: xm3 = x3_rows(t) * simrep3  (f32r out)   x3 passes t=0,1,2
     - PE : psum[32, 2048] += w3[:, t].T @ xm3       (K=96, N=512 x4, f32r)
     - ACT: psum -> sbuf f32; DMA out.
"""
import sys

import numpy as np

sys.path.insert(0, "/opt/trn_rl_repo")

import concourse.bass as bass
import concourse.mybir as mybir
from concourse.bass_utils import run_bass_kernel_spmd

F32 = mybir.dt.float32
F32R = mybir.dt.float32r
BF16 = mybir.dt.bfloat16
EXP = mybir.ActivationFunctionType.Exp

B, C, H, W = 4, 32, 256, 256
O = 32
ALPHA = 8.3
R = 128  # output rows per core
WP = W + 2  # padded width
FR = R + 2  # frame rows per core
NPIX = R * W  # 32768
TROWS = 8  # rows per tile
TILE = TROWS * W  # 2048
NT = R // TROWS  # 16
CH_ROWS = TROWS + 2  # x3 chunk rows
MMN = 512  # matmul free-dim chunk
QN = TILE // MMN  # 4


def build_nc():
    nc = bass.Bass("TRN2", target_bir_lowering=False, debug=False, num_devices=8)
    x3_in = nc.declare_dram_parameter("x3", [96, FR * WP], F32, isOutput=False)
    dp_in = nc.declare_dram_parameter("dp", [FR, WP], F32, isOutput=False)
    w3_in = nc.declare_dram_parameter("w3", [96, 96], F32, isOutput=False)
    out_d = nc.declare_dram_parameter("out", [O, NPIX], F32, isOutput=True)
    simd = nc.dram_tensor("simd", [9, NPIX], BF16)
    simd_r = simd.ap().rearrange("k (r w) -> k r w", r=R)

    from contextlib import ExitStack

    ctx = ExitStack()
    with ctx:
        d_sb = ctx.enter_context(nc.sbuf_tensor([128, 3 * WP], F32))
        adiff9 = ctx.enter_context(nc.sbuf_tensor([128, 9 * W], F32))
        sim9 = ctx.enter_context(nc.sbuf_tensor([128, 9 * W], BF16))
        w3_sb = ctx.enter_context(nc.sbuf_tensor([96, 96], F32))
        w3r = ctx.enter_context(nc.sbuf_tensor([96, 96], F32R))
        x3c = ctx.enter_context(nc.sbuf_tensor([96, 2 * CH_ROWS * WP], F32))
        simrep3 = ctx.enter_context(nc.sbuf_tensor([96, 2 * TILE], BF16))
        xm3 = ctx.enter_context(nc.sbuf_tensor([96, 2 * TILE], F32R))
        out_sb = ctx.enter_context(nc.sbuf_tensor([32, 2 * TILE], F32))
        psum = ctx.enter_context(nc.psum_tensor([32, 2 * TILE], F32))
        ld_sem = ctx.enter_context(nc.semaphore("ld_sem"))
        x_e = ctx.enter_context(nc.semaphore("x_e"))
        x_o = ctx.enter_context(nc.semaphore("x_o"))
        sim_dve = ctx.enter_context(nc.semaphore("sim_dve"))
        act_exp = ctx.enter_context(nc.semaphore("act_exp"))
        sim_st = ctx.enter_context(nc.semaphore("sim_st"))
        bc_e = ctx.enter_context(nc.semaphore("bc_e"))
        bc_o = ctx.enter_context(nc.semaphore("bc_o"))
        mod_sem = ctx.enter_context(nc.semaphore("mod_sem"))
        wr_sem = ctx.enter_context(nc.semaphore("wr_sem"))
        pe_sem = ctx.enter_context(nc.semaphore("pe_sem"))
        act_cp = ctx.enter_context(nc.semaphore("act_cp"))
        st_e = ctx.enter_context(nc.semaphore("st_e"))
        st_o = ctx.enter_context(nc.semaphore("st_o"))
        block = ctx.enter_context(nc.Block())

        @block.sync
        def _(sync: bass.BassEngine):
            # startup loads: d (3 row-shifted views), w3
            for t in range(3):
                sync.dma_start(
                    d_sb[:, t * WP : (t + 1) * WP], dp_in[t : t + 128, :]
                ).then_inc(ld_sem, 16)
            sync.dma_start(w3_sb[:], w3_in[:]).then_inc(ld_sem, 16)
            # sim -> DRAM
            for k in range(9):
                sync.wait_ge(act_exp, k + 1)
                sync.dma_start(
                    simd_r[k], sim9[:, k * W : (k + 1) * W]
                ).then_inc(sim_st, 16)
            # main loop
            for i in range(NT):
                bi = i % 2
                # x3 chunk for tile i
                if i >= 2:
                    sync.wait_ge(mod_sem, 3 * (i - 2) + 3)
                sync.dma_start(
                    x3c[:, bi * CH_ROWS * WP : (bi + 1) * CH_ROWS * WP],
                    x3_in[:, i * TROWS * WP : (i * TROWS + CH_ROWS) * WP],
                ).then_inc(x_e if bi == 0 else x_o, 16)
                # broadcast sim rows for the 3 passes
                if i == 0:
                    sync.wait_ge(sim_st, 9 * 16)
                for t in range(3):
                    s = 3 * i + t
                    sb = s % 2
                    if s >= 2:
                        sync.wait_ge(mod_sem, s - 1)
                    for j in range(3):
                        sync.dma_start(
                            simrep3[
                                32 * j : 32 * (j + 1),
                                sb * TILE : (sb + 1) * TILE,
                            ],
                            simd[
                                3 * t + j : 3 * t + j + 1,
                                i * TILE : (i + 1) * TILE,
                            ].to_broadcast((32, TILE)),
                        ).then_inc(bc_e if sb == 0 else bc_o, 16)
                # store tile i-1
                if i >= 1:
                    sync.wait_ge(act_cp, i)
                    sync.dma_start(
                        out_d[:, (i - 1) * TILE : i * TILE],
                        out_sb[:, ((i - 1) % 2) * TILE : ((i - 1) % 2 + 1) * TILE],
                    ).then_inc(st_e if (i - 1) % 2 == 0 else st_o, 16)
            sync.wait_ge(act_cp, NT)
            sync.dma_start(
                out_d[:, (NT - 1) * TILE :],
                out_sb[:, ((NT - 1) % 2) * TILE : ((NT - 1) % 2 + 1) * TILE],
            ).then_inc(st_e if (NT - 1) % 2 == 0 else st_o, 16)

        @block.vector
        def _(vector):
            # sim phase: diff + abs per tap
            vector.wait_ge(ld_sem, 64)
            for t in range(3):
                for j in range(3):
                    k = 3 * t + j
                    vector.tensor_sub(
                        adiff9[:, k * W : (k + 1) * W],
                        d_sb[:, WP + 1 : WP + 1 + W],
                        d_sb[:, t * WP + j : t * WP + j + W],
                    )
                    vector.drain()
                    vector.scalar_tensor_tensor(
                        adiff9[:, k * W : (k + 1) * W],
                        adiff9[:, k * W : (k + 1) * W],
                        -1.0,
                        adiff9[:, k * W : (k + 1) * W],
                        op0=mybir.AluOpType.mult,
                        op1=mybir.AluOpType.max,
                    ).then_inc(sim_dve, 1)
            # round weights to f32r
            vector.wait_ge(ld_sem, 64)
            vector.tensor_copy(w3r[:], w3_sb[:]).then_inc(wr_sem, 1)
            # modulation loop
            for i in range(NT):
                bi = i % 2
                vector.wait_ge(x_e if bi == 0 else x_o, 16 * (i // 2 + 1))
                for t in range(3):
                    s = 3 * i + t
                    sb = s % 2
                    vector.wait_ge(bc_e if sb == 0 else bc_o, 48 * (s // 2 + 1))
                    if s >= 2:
                        vector.wait_ge(pe_sem, s - 1)
                    xv = x3c[:, bi * CH_ROWS * WP : (bi + 1) * CH_ROWS * WP]
                    xv = xv.rearrange("p (r w) -> p r w", w=WP)
                    vector.tensor_mul(
                        xm3[:, sb * TILE : (sb + 1) * TILE].rearrange(
                            "p (r w) -> p r w", w=W
                        ),
                        xv[:, t : t + TROWS, 1 : 1 + W],
                        simrep3[:, sb * TILE : (sb + 1) * TILE].rearrange(
                            "p (r w) -> p r w", w=W
                        ),
                    ).then_inc(mod_sem, 1)

        @block.tensor
        def _(tensor):
            tensor.wait_ge(wr_sem, 1)
            for i in range(NT):
                bi = i % 2
                if i >= 2:
                    tensor.wait_ge(act_cp, i - 1)
                for t in range(3):
                    s = 3 * i + t
                    sb = s % 2
                    tensor.wait_ge(mod_sem, s + 1)
                    for q in range(QN):
                        mm = tensor.matmul(
                            psum[:, bi * TILE + q * MMN : bi * TILE + (q + 1) * MMN],
                            w3r[:, 32 * t : 32 * (t + 1)],
                            xm3[:, sb * TILE + q * MMN : sb * TILE + (q + 1) * MMN],
                            start=(t == 0),
                            stop=(t == 2),
                        )
                        if q == QN - 1:
                            mm.then_inc(pe_sem, 1)

        @block.scalar
        def _(scalar):
            # exp per tap (bf16 out)
            for k in range(9):
                scalar.wait_ge(sim_dve, k + 1)
                scalar.activation(
                    sim9[:, k * W : (k + 1) * W],
                    adiff9[:, k * W : (k + 1) * W],
                    EXP,
                    scale=-ALPHA,
                ).then_inc(act_exp, 1)
            # psum -> sbuf copies
            for i in range(NT):
                bi = i % 2
                scalar.wait_ge(pe_sem, 3 * i + 3)
                if i >= 2:
                    scalar.wait_ge(st_e if i % 2 == 0 else st_o, 16 * (i // 2))
                scalar.copy(
                    out_sb[:, bi * TILE : (bi + 1) * TILE],
                    psum[:, bi * TILE : (bi + 1) * TILE],
                ).then_inc(act_cp, 1)

    return nc


_NC_CACHE = None


def _get_nc():
    global _NC_CACHE
    if _NC_CACHE is None:
        _NC_CACHE = build_nc()
    return _NC_CACHE


def _prep_core(x, depth, weight, core):
    b, half = core // 2, core % 2
    r0 = half * R
    # padded frame [C, FR, WP]: image rows r0-1 .. r0+R, zero-padded
    xpad = np.zeros((C, FR, WP), dtype=np.float32)
    dpad = np.zeros((FR, WP), dtype=np.float32)
    lo, hi = r0 - 1, r0 + R + 1
    slo, shi = max(lo, 0), min(hi, H)
    xpad[:, slo - lo : shi - lo, 1 : 1 + W] = x[b, :, slo:shi, :]
    dpad[slo - lo : shi - lo, 1 : 1 + W] = depth[b, 0, slo:shi, :]
    # x3: 3 column-shift blocks stacked on partitions
    x3 = np.zeros((3, C, FR, WP), dtype=np.float32)
    x3[0, :, :, 1:] = xpad[:, :, :-1]  # j=0: w-1
    x3[1] = xpad  # j=1: w
    x3[2, :, :, :-1] = xpad[:, :, 1:]  # j=2: w+1
    return {
        "x3": x3.reshape(3 * C, FR * WP),
        "dp": dpad,
        "w3": None,  # filled by caller (shared)
    }


def kernel(x, depth, weight):
    x = np.ascontiguousarray(x, dtype=np.float32)
    depth = np.ascontiguousarray(depth, dtype=np.float32)
    weight = np.ascontiguousarray(weight, dtype=np.float32)

    # w3[32j + c, 32t + o] = weight[o, c, t, j]
    w3 = np.transpose(weight, (3, 1, 2, 0)).reshape(96, 96).copy()

    in_maps = []
    for core in range(8):
        m = _prep_core(x, depth, weight, core)
        m["w3"] = w3
        in_maps.append(m)

    nc = _get_nc()
    res = run_bass_kernel_spmd(nc, in_maps, list(range(8)))

    out = np.empty((B, O, H, W), dtype=np.float32)
    for core in range(8):
        b, half = core // 2, core % 2
        out[b, :, half * R : (half + 1) * R, :] = res.results[core]["out"].reshape(
            O, R, W
        )
    return out



# revision 3
# speedup vs baseline: 1.3277x; 1.3277x over previous
"""Depth-aware 3x3 convolution on 8 Trainium2 NeuronCores (Bass).

out[b,o,h,w] = sum_{c,t,j} weight[o,c,t,j] * x[b,c,h+t-1,w+j-1]
                           * exp(-8.3*|depth[b,h,w] - depth[b,h+t-1,w+j-1]|)

Sharding: core = 2*b + (h >= 128); each core computes a [32, 128, 256] slab.

Per-core pipeline (bf16 working dtypes, f32 psum):
  sim head (pipelined by column-tap group j):
    depth row-views (f32) -> DVE sub -> ACT |.| -> ACT exp(-a*.) bf16
    -> DRAM simd -> per-tile-pair DMA broadcast to 32 partitions/group.
  steady state, 4 pairs of 16-row tiles:
    DVE: 3 modulate passes/tile, xm[96,4128] = x3_chunk * simrep
         (contiguous bf16 tensor_tensor -> 2x perf mode),
    PE : per pass 4 chunks x 2 col-groups (tile_position) of K=96 N=512
         matmuls accumulating psum[64, 2048],
    ACT: psum -> bf16 out_sb copy; output stored in a raw device layout
         decoded on host.

Key layout/DMA choices (vs a naive implementation):
  - x3 = three column-pre-shifted copies of x (host-built, bf16) so every
    DVE/PE access is 4-byte aligned and row-tap shifts come free via AP
    row offsets.
  - sim is broadcast once per tile PAIR per j-group (3 big DMAs instead of
    9 small ones); the center tap (sim==1) is never shipped - its simrep
    block is pre-filled with ones at startup.
  - broadcasts ride the SP HWDGE ring while loads/stores ride the ACT ring.
  - head-phase buffers (adiff, d_sb, sim_sb) alias simrep slot 2's column
    space: broadcasts only touch slot 2 after the sim store, when all
    three are dead.

The builder takes reps=N to chain N back-to-back executions of the whole
pipeline inside one NEFF (used by the benchmark harness to measure pure
device time via a reps slope; all semaphore thresholds are global-indexed).
"""
import sys

import numpy as np

sys.path.insert(0, "/opt/trn_rl_repo")

import concourse.bass as bass
import concourse.mybir as mybir
from concourse.bass_utils import run_bass_kernel_spmd

F32 = mybir.dt.float32
BF16 = mybir.dt.bfloat16
NPBF16 = mybir.dt.np(mybir.dt.bfloat16)
EXP = mybir.ActivationFunctionType.Exp
ABS = mybir.ActivationFunctionType.Abs

B, C, H, W = 4, 32, 256, 256
O = 32
ALPHA = 8.3
BIG = 60.0
R = 128
WP = W + 2
FR = R + 2
TROWS = 16
NT = R // TROWS  # 8 tiles
NP_ = NT // 2  # 4 pairs
PCH = 2 * TROWS + 2  # 34
PASS_FREE = TROWS * WP
TBLK = 2 * TROWS * WP  # simrep t-block (32 rows)
PAIR_SIM = 3 * TBLK
NSIM = 9 * WP


def build_nc(reps=1):
    nc = bass.Bass("TRN2", target_bir_lowering=False, debug=False, num_devices=8)
    x3_in = nc.declare_dram_parameter("x3", [96, FR * WP], BF16, isOutput=False)
    dp_in = nc.declare_dram_parameter("dp", [FR, WP], F32, isOutput=False)
    w3_in = nc.declare_dram_parameter("w3", [96, 96], BF16, isOutput=False)
    oo_in = nc.declare_dram_parameter("oo", [1, TBLK], BF16, isOutput=False)
    out_d = nc.declare_dram_parameter("out", [64, NT * 2048], BF16, isOutput=True)
    simd = nc.dram_tensor("simd", [9, R * WP], BF16)
    simd_jtrq = simd.ap().rearrange("(t j) (r q) -> j t r q", t=3, r=R)
    simd_rkq = simd.ap().rearrange("k (r q) -> r k q", r=R)
    _simd_rtjq = simd.ap().rearrange("(t j) (r q) -> r t j q", t=3, r=R)
    simd_rkq_j = [_simd_rtjq[:, :, j] for j in range(3)]

    from contextlib import ExitStack

    ctx = ExitStack()
    with ctx:
        w3_sb = ctx.enter_context(nc.sbuf_tensor([96, 96], BF16))
        x3c = ctx.enter_context(nc.sbuf_tensor([96, 2 * PCH * WP], BF16))
        simrep = ctx.enter_context(nc.sbuf_tensor([128, 3 * PAIR_SIM], BF16))
        xm = ctx.enter_context(nc.sbuf_tensor([96, 2 * PASS_FREE], BF16))
        out_sb = ctx.enter_context(nc.sbuf_tensor([64, 2 * 2048], BF16))
        psum = ctx.enter_context(nc.psum_tensor([64, 2 * 2048], F32))

        # head-phase buffers aliased into simrep slot 2 (see module docstring):
        #   t0 block: adiff [128, 9*WP] f32 + d_sb [128, 3*WP+4] f32
        #   t2 block: sim_sb [128, 9*WP] bf16
        slot2b = simrep.ap()[:, 2 * PAIR_SIM : 3 * PAIR_SIM]
        f32v = slot2b.bitcast(F32)
        adiff = f32v[:, 0:NSIM]
        d_sb = f32v[:, NSIM : NSIM + 3 * WP + 4]
        sim_sb = slot2b[:, 2 * TBLK : 2 * TBLK + NSIM]

        ld = ctx.enter_context(nc.semaphore("ld"))
        sub_s = ctx.enter_context(nc.semaphore("sub_s"))
        act_sim = ctx.enter_context(nc.semaphore("act_sim"))
        sim_st = ctx.enter_context(nc.semaphore("sim_st"))
        bc = ctx.enter_context(nc.semaphore("bc"))
        x_e = ctx.enter_context(nc.semaphore("x_e"))
        x_o = ctx.enter_context(nc.semaphore("x_o"))
        mod = ctx.enter_context(nc.semaphore("mod"))
        pe = ctx.enter_context(nc.semaphore("pe"))
        cp = ctx.enter_context(nc.semaphore("cp"))
        st = ctx.enter_context(nc.semaphore("st"))
        block = ctx.enter_context(nc.Block())

        xmv = xm.ap().rearrange("p (s r q) -> p s r q", s=2, q=WP)
        psv = psum.ap().rearrange("p (b r w) -> p b r w", b=2, w=W)
        srv = simrep.ap().rearrange("p (s t r q) -> p s t r q", s=3, t=3, q=WP)

        def x_sem(gp):
            return x_e if gp % 2 == 0 else x_o

        def x3_load(eng, gp, i0):
            eng.dma_start(
                x3c[:, (gp % 2) * PCH * WP : (gp % 2 + 1) * PCH * WP],
                x3_in[:, (TROWS * i0) * WP : (TROWS * i0 + PCH) * WP],
            ).then_inc(x_sem(gp), 16)

        @block.sync
        def _(sync: bass.BassEngine):
            # one-time: ones into the center-tap (j=1, t=1) region of slots 0/1
            # (slot 2's goes in later, once the head-phase aliases are dead)
            for s in range(2):
                sync.dma_start(
                    simrep[32:64, s * PAIR_SIM + TBLK : s * PAIR_SIM + 2 * TBLK],
                    oo_in[0:1, :].to_broadcast((32, TBLK)),
                ).then_inc(bc, 16)
            for rep in range(reps):
                p0 = rep * NP_
                for p in range(NP_):
                    gp = p0 + p
                    slot = gp % 3
                    if gp >= 3:
                        sync.wait_ge(mod, 6 * gp - 12)
                    if gp == 2:
                        sync.dma_start(
                            simrep[
                                32:64, 2 * PAIR_SIM + TBLK : 2 * PAIR_SIM + 2 * TBLK
                            ],
                            oo_in[0:1, :].to_broadcast((32, TBLK)),
                        ).then_inc(bc, 16)
                    for j in range(3):
                        sync.wait_ge(sim_st, 48 * rep + 16 * (j + 1))
                        if j == 1:
                            sync.dma_start(
                                srv[32:64, slot, 0::2],
                                simd_jtrq[
                                    1:2, 0::2, 2 * TROWS * p : 2 * TROWS * (p + 1), :
                                ].to_broadcast((32, 2, 2 * TROWS, WP)),
                            ).then_inc(bc, 16)
                        else:
                            sync.dma_start(
                                srv[32 * j : 32 * (j + 1), slot],
                                simd_jtrq[
                                    j : j + 1, :, 2 * TROWS * p : 2 * TROWS * (p + 1), :
                                ].to_broadcast((32, 3, 2 * TROWS, WP)),
                            ).then_inc(bc, 16)

        @block.vector
        def _(vector):
            for rep in range(reps):
                t0 = rep * NT
                vector.wait_ge(ld, 64 * rep + 48)
                if rep > 0:
                    vector.wait_ge(act_sim, 3 * rep)
                for j in range(3):
                    for t in range(3):
                        k = 3 * t + j
                        vector.tensor_sub(
                            adiff[:, k * WP : (k + 1) * WP],
                            d_sb[:, WP + 2 : 2 * WP + 2],
                            d_sb[:, 1 + t * WP + j : 1 + t * WP + j + WP],
                        ).then_inc(sub_s, 1)
                for i in range(NT):
                    gi = t0 + i
                    gp, a = gi // 2, gi % 2
                    bi = gp % 2
                    si = gp % 3
                    if a == 0:
                        vector.wait_ge(x_sem(gp), 16 * (gp // 2 + 1))
                        ones_extra = 32 if gp < 2 else 48
                        vector.wait_ge(bc, ones_extra + 48 * (gp + 1))
                    for t in range(3):
                        gs = 3 * gi + t
                        sb = gs % 2
                        if gs >= 2:
                            vector.wait_ge(pe, gs - 1)
                        vector.tensor_mul(
                            xm[:, sb * PASS_FREE : (sb + 1) * PASS_FREE],
                            x3c[
                                :,
                                bi * PCH * WP
                                + (16 * a + t) * WP : bi * PCH * WP
                                + (16 * a + t) * WP
                                + PASS_FREE,
                            ],
                            simrep[
                                0:96,
                                si * PAIR_SIM
                                + t * TBLK
                                + 16 * a * WP : si * PAIR_SIM
                                + t * TBLK
                                + 16 * a * WP
                                + PASS_FREE,
                            ],
                        ).then_inc(mod, 1)

        @block.tensor
        def _(tensor):
            for rep in range(reps):
                t0 = rep * NT
                tensor.wait_ge(ld, 64 * (rep + 1))
                for i in range(NT):
                    gi = t0 + i
                    bi = gi % 2
                    if gi >= 2:
                        tensor.wait_ge(cp, gi - 1)
                    for t in range(3):
                        gs = 3 * gi + t
                        sb = gs % 2
                        tensor.wait_ge(mod, gs + 1)
                        for c4 in range(4):
                            for g in range(2):
                                mm = tensor.matmul(
                                    psv[
                                        32 * g : 32 * (g + 1),
                                        bi,
                                        2 * c4 : 2 * c4 + 2,
                                        :,
                                    ],
                                    w3_sb[:, 32 * t : 32 * (t + 1)],
                                    xmv[:, sb, 8 * g + 2 * c4 : 8 * g + 2 * c4 + 2, 0:W],
                                    start=(t == 0),
                                    stop=(t == 2),
                                )
                                if c4 == 3 and g == 1:
                                    mm.then_inc(pe, 1)

        @block.scalar
        def _(scalar):
            for rep in range(reps):
                t0 = rep * NT
                p0 = rep * NP_
                if rep > 0:
                    scalar.wait_ge(pe, 24 * rep)
                for t in range(3):
                    scalar.dma_start(
                        d_sb[:, 1 + t * WP : 1 + (t + 1) * WP], dp_in[t : t + 128, :]
                    ).then_inc(ld, 16)
                scalar.dma_start(w3_sb[:], w3_in[:]).then_inc(ld, 16)
                for p in range(2):
                    gp = p0 + p
                    if gp >= 2:
                        scalar.wait_ge(mod, 6 * gp - 6)
                    x3_load(scalar, gp, 2 * p)
                if rep > 0:
                    scalar.wait_ge(sim_st, 48 * rep)
                    scalar.wait_ge(bc, 48 + 192 * rep)
                for j in range(3):
                    # taps (t, j) for t=0..2: adiff cols {(3t+j)*WP}
                    jcols = adiff.rearrange("p (t3 j3 q) -> p t3 j3 q", t3=3, q=WP)[
                        :, :, j
                    ]
                    scols = sim_sb.rearrange("p (t3 j3 q) -> p t3 j3 q", t3=3, q=WP)[
                        :, :, j
                    ]
                    scalar.wait_ge(sub_s, 9 * rep + 3 * (j + 1))
                    scalar.activation(jcols, jcols, ABS)
                    scalar.activation(scols, jcols, EXP, scale=-ALPHA).then_inc(
                        act_sim, 1
                    )
                    scalar.wait_ge(act_sim, 3 * rep + j + 1)
                    scalar.dma_start(
                        simd_rkq_j[j], scols
                    ).then_inc(sim_st, 16)
                for i in range(NT):
                    gi = t0 + i
                    gp, a = gi // 2, gi % 2
                    bi = gi % 2
                    scalar.wait_ge(pe, 3 * gi + 3)
                    if gi >= 2:
                        scalar.wait_ge(st, 16 * (gi - 1))
                    scalar.copy(
                        out_sb[:, bi * 2048 : (bi + 1) * 2048],
                        psum[:, bi * 2048 : (bi + 1) * 2048],
                    ).then_inc(cp, 1)
                    scalar.wait_ge(cp, gi + 1)
                    scalar.dma_start(
                        out_d[:, i * 2048 : (i + 1) * 2048],
                        out_sb[:, bi * 2048 : (bi + 1) * 2048],
                    ).then_inc(st, 16)
                    if a == 1 and i // 2 + 2 < NP_:
                        gp2 = gp + 2
                        scalar.wait_ge(mod, 6 * gp2 - 6)
                        x3_load(scalar, gp2, 2 * (i // 2 + 2))

    return nc


_NC_CACHE = {}


def _get_nc(reps=1):
    if reps not in _NC_CACHE:
        _NC_CACHE[reps] = build_nc(reps)
    return _NC_CACHE[reps]


def _prep_core(x, depth, core):
    b, half = core // 2, core % 2
    r0 = half * R
    lo, hi = r0 - 1, r0 + R + 1
    slo, shi = max(lo, 0), min(hi, H)
    xpad = np.zeros((C, FR, WP), dtype=np.float32)
    xpad[:, slo - lo : shi - lo, 1 : 1 + W] = x[b, :, slo:shi, :]
    dpad = np.full((FR, WP), BIG, dtype=np.float32)
    dpad[slo - lo : shi - lo, 1 : 1 + W] = depth[b, 0, slo:shi, :]
    x3 = np.zeros((3, C, FR, WP), dtype=np.float32)
    x3[0] = xpad
    x3[1][:, :, :-1] = xpad[:, :, 1:]
    x3[2][:, :, :-2] = xpad[:, :, 2:]
    return {
        "x3": x3.reshape(96, FR * WP).astype(NPBF16),
        "dp": dpad,
    }


def _prep_maps(inputs):
    x = np.ascontiguousarray(inputs["x"], dtype=np.float32)
    depth = np.ascontiguousarray(inputs["depth"], dtype=np.float32)
    weight = np.ascontiguousarray(inputs["weight"], dtype=np.float32)
    w3 = np.transpose(weight, (3, 1, 2, 0)).reshape(96, 96).astype(NPBF16)
    oo = np.ones((1, TBLK), dtype=NPBF16)
    in_maps = []
    for core in range(8):
        m = _prep_core(x, depth, core)
        m["w3"] = w3
        m["oo"] = oo
        in_maps.append(m)
    return in_maps


def _decode_out(raw):
    t = raw.astype(np.float32).reshape(2, 32, NT, 8, W)
    return t.transpose(1, 2, 0, 3, 4).reshape(O, R, W)


def kernel(x, depth, weight):
    in_maps = _prep_maps({"x": x, "depth": depth, "weight": weight})
    nc = _get_nc()
    res = run_bass_kernel_spmd(nc, in_maps, list(range(8)))
    out = np.empty((B, O, H, W), dtype=np.float32)
    for core in range(8):
        b, half = core // 2, core % 2
        out[b, :, half * R : (half + 1) * R, :] = _decode_out(res.results[core]["out"])
    return out
